# revision 1
# baseline (speedup 1.0000x reference)
"""Trainium2 Bass kernel for LoFTR-style linear attention (nn_AttentionLayer).

Data-parallel over B=1024 across 8 NeuronCores (128 batches/core, no
collectives).  All heavy compute in bf16 on the TensorEngine with fp32 PSUM
accumulation.

Key tricks:
  - Host-side transpose: pass xg^T (768, 16384) so projections need no
    on-chip transpose (contraction dim = input features on partitions).
  - phi(x) = elu(x)+1 == max(exp(min(x,0)), 1+x): computed with 2 ACT passes
    (Relu(-z), Exp(-u)) + 1 fused DVE scalar_tensor_tensor (max(z+1, e)).
  - V augmented with a ones column so KV' = phi(K)^T [V|1] yields both KV and
    Ksum in one matmul; U = phi(Q) @ KV' yields both the unnormalized output
    and the normalizer denominator in one matmul.
  - Q kept feature-on-partition (bias via ACT per-partition operand);
    K/V token-on-partition (bias folded into the matmul as a K=1 ones-row).
  - Attention einsums packed 2 heads per matmul via PE row/col tile_position.
"""

import numpy as np
import ml_dtypes

NCORES = 8
B, L, HID, GUID, H, D = 1024, 128, 512, 256, 8, 64
KIN = HID + GUID          # 768
BSH = B // NCORES         # 128 batches per core
TOK = BSH * L             # 16384 tokens per core
BLK = 512                 # tokens per block (4 batches)
EPS = 1e-6

_CACHE = {}


def _build(nblocks):
    from contextlib import ExitStack
    import concourse.bass as bass
    import concourse.mybir as mybir
    import concourse.tile as tile
    from concourse import bacc

    f32 = mybir.dt.float32
    bf16 = mybir.dt.bfloat16
    AF = mybir.ActivationFunctionType
    OP = mybir.AluOpType

    ntok = nblocks * BLK

    nc = bacc.Bacc("TRN2", target_bir_lowering=False, debug=False,
                   num_devices=NCORES)

    xgT = nc.dram_tensor("xgT", (KIN, ntok), bf16, kind="ExternalInput").ap()
    wq_d = nc.dram_tensor("wq", (128, 6, 512), bf16, kind="ExternalInput").ap()
    wk_d = nc.dram_tensor("wk", (128, 6, 512), bf16, kind="ExternalInput").ap()
    wv_d = nc.dram_tensor("wv", (128, 4, 512), bf16, kind="ExternalInput").ap()
    wk7_d = nc.dram_tensor("wk7", (128, 512), bf16, kind="ExternalInput").ap()
    bvb_d = nc.dram_tensor("bvb", (128, 8, 64), bf16, kind="ExternalInput").ap()
    qb_d = nc.dram_tensor("qb", (128, 4, 2), f32, kind="ExternalInput").ap()
    out_d = nc.dram_tensor("out", (ntok, 512), f32, kind="ExternalOutput").ap()

    with tile.TileContext(nc) as tc, ExitStack() as ctx:
        consts = ctx.enter_context(tc.tile_pool(name="consts", bufs=1))
        xg_pool = ctx.enter_context(tc.tile_pool(name="xg", bufs=3))
        qphi_pool = ctx.enter_context(tc.tile_pool(name="qphi", bufs=2))
        tmp_pool = ctx.enter_context(tc.tile_pool(name="tmp", bufs=4))
        kphi_pool = ctx.enter_context(tc.tile_pool(name="kphi", bufs=2))
        vp_pool = ctx.enter_context(tc.tile_pool(name="vp", bufs=2))
        kv_pool = ctx.enter_context(tc.tile_pool(name="kv", bufs=2))
        out_pool = ctx.enter_context(tc.tile_pool(name="outp", bufs=4))
        rcp_pool = ctx.enter_context(tc.tile_pool(name="rcp", bufs=4))
        psq_pool = ctx.enter_context(tc.tile_pool(name="psq", bufs=2, space="PSUM"))
        pskv_pool = ctx.enter_context(tc.tile_pool(name="pskv", bufs=3, space="PSUM"))
        psatt_pool = ctx.enter_context(tc.tile_pool(name="psatt", bufs=1, space="PSUM"))
        psu_pool = ctx.enter_context(tc.tile_pool(name="psu", bufs=2, space="PSUM"))

        wq_t = consts.tile([128, 6, 512], bf16)
        wk_t = consts.tile([128, 6, 512], bf16)
        wv_t = consts.tile([128, 4, 512], bf16)
        wk7_t = consts.tile([128, 512], bf16)
        bvb_t = consts.tile([128, 8, 64], bf16)
        qb_t = consts.tile([128, 4, 2], f32)
        pad_t = consts.tile([128, 128], bf16)
        # weight loads split per k-slice so the first matmuls start early and
        # each transfer lands on its own DMA queue
        for k in range(6):
            nc.sync.dma_start(wq_t[:, k, :], wq_d[:, k, :])
            nc.sync.dma_start(wk_t[:, k, :], wk_d[:, k, :])
            if k < 4:
                nc.sync.dma_start(wv_t[:, k, :], wv_d[:, k, :])
        nc.sync.dma_start(wk7_t[:], wk7_d[:])
        nc.sync.dma_start(bvb_t[:], bvb_d[:])
        nc.sync.dma_start(qb_t[:], qb_d[:])
        # pad_t.T @ wk7 adds the bk bias row to every token: row 0 is ones,
        # rows 1..127 zero; wk7 row 0 holds bk.
        nc.vector.memset(pad_t[:], 0.0)
        nc.vector.memset(pad_t[0:1, :], 1.0)

        # KV' staging tiles with statically-zeroed dual halves (workaround:
        # matmuls with lhsT/rhs at SBUF base partition 64 crash, so U matmuls
        # run full K=128 against rhs whose other-dual rows are zero).
        # Two copies each for manual double-buffering across batches.
        kvE = [consts.tile([128, 4, 65], bf16, name=f"kvE{i}") for i in range(2)]
        kvO = [consts.tile([128, 4, 65], bf16, name=f"kvO{i}") for i in range(2)]
        for i in range(2):
            nc.vector.memset(kvE[i][64:128, :, :], 0.0)
            nc.vector.memset(kvO[i][0:64, :, :], 0.0)

        for j in range(nblocks):
            xg_t = xg_pool.tile([128, 6, 512], bf16, tag="xg")
            xgv = xgT[:, j * BLK:(j + 1) * BLK].rearrange(
                "(ko p) n -> p ko n", p=128)
            for k in range(6):
                nc.sync.dma_start(xg_t[:, k, :], xgv[:, k, :])

            # ---- Q projection (feature-on-partition) + phi ----
            qphi_t = qphi_pool.tile([128, 4, 512], bf16, tag="qphi")
            for m in range(4):
                ps = psq_pool.tile([128, 512], f32, tag="psq")
                for k in range(6):
                    nc.tensor.matmul(
                        ps[:],
                        wq_t[:, k, m * 128:(m + 1) * 128],
                        xg_t[:, k, :],
                        start=(k == 0), stop=(k == 5),
                    )
                u = tmp_pool.tile([128, 512], f32, tag="tmp")
                nc.scalar.activation(u[:], ps[:], AF.Relu,
                                     bias=qb_t[:, m, 0:1], scale=-1.0)
                e = tmp_pool.tile([128, 512], f32, tag="tmp")
                nc.scalar.activation(e[:], u[:], AF.Exp, scale=-1.0)
                nc.vector.scalar_tensor_tensor(
                    qphi_t[:, m, :], ps[:], qb_t[:, m, 1:2], e[:],
                    OP.add, OP.max,
                )

            for bi in range(4):
                b = j * 4 + bi
                bs = slice(bi * 128, (bi + 1) * 128)

                # ---- K projection (token-on-partition), bias via ones-row ----
                ps_k = pskv_pool.tile([128, 512], f32, tag="pskv")
                for k in range(6):
                    nc.tensor.matmul(ps_k[:], xg_t[:, k, bs], wk_t[:, k, :],
                                     start=(k == 0), stop=False)
                nc.tensor.matmul(ps_k[:], pad_t[:], wk7_t[:],
                                 start=False, stop=True)

                # ---- V projection (bias folded into the V' copy below) ----
                ps_v = pskv_pool.tile([128, 512], f32, tag="pskv")
                for k in range(4):
                    nc.tensor.matmul(ps_v[:], xg_t[:, k, bs], wv_t[:, k, :],
                                     start=(k == 0), stop=(k == 3))

                # ---- phi(K) ----
                u = tmp_pool.tile([128, 512], f32, tag="tmp")
                nc.scalar.activation(u[:], ps_k[:], AF.Relu, scale=-1.0)
                e = tmp_pool.tile([128, 512], f32, tag="tmp")
                nc.scalar.activation(e[:], u[:], AF.Exp, scale=-1.0)
                kphi_t = kphi_pool.tile([128, 512], bf16, tag="kphi")
                nc.vector.scalar_tensor_tensor(
                    kphi_t[:], ps_k[:], 1.0, e[:], OP.add, OP.max)

                # ---- V' = [V + bv | 1] per head ----
                vp_t = vp_pool.tile([128, 8, 65], bf16, tag="vp")
                nc.vector.memset(vp_t[:, :, 64:65], 1.0)
                nc.vector.tensor_tensor(
                    vp_t[:, :, 0:64],
                    ps_v[:].rearrange("p (h d) -> p h d", d=64),
                    bvb_t[:], OP.add)

                # ---- KV' = phi(K)^T @ V' : 2 heads per pair via col tiling ----
                ps_kv_full = psatt_pool.tile([128, 512], f32, tag="psatt",
                                             name="ps_kv")
                ps_kv = ps_kv_full[:, :260]
                for p in range(4):
                    nc.tensor.matmul(
                        ps_kv[0:64, p * 65:(p + 1) * 65],
                        kphi_t[:, p * 128:p * 128 + 64],
                        vp_t[:, 2 * p, :],
                        start=True, stop=True, tile_position=(0, 0))
                    nc.tensor.matmul(
                        ps_kv[64:128, p * 65:(p + 1) * 65],
                        kphi_t[:, p * 128 + 64:(p + 1) * 128],
                        vp_t[:, 2 * p + 1, :],
                        start=True, stop=True, tile_position=(0, 64))
                kvE_t = kvE[b % 2]
                kvO_t = kvO[b % 2]
                # split across ACT and DVE so the two copies run in parallel
                nc.scalar.copy(
                    kvE_t[0:64, :, :],
                    ps_kv[0:64, :].rearrange("p (c j) -> p c j", j=65))
                nc.vector.tensor_copy(
                    kvO_t[64:128, :, :],
                    ps_kv[64:128, :].rearrange("p (c j) -> p c j", j=65))

                # ---- U = phi(Q) @ KV' : full K=128 against zero-padded KV ----
                out_t = out_pool.tile([128, 512], f32, tag="outp")
                for half in range(2):
                    ps_u_full = psu_pool.tile([128, 512], f32, tag="psu",
                                              name="ps_u")
                    ps_u = ps_u_full[:, :260]
                    for pp in range(2):
                        p = half * 2 + pp
                        nc.tensor.matmul(
                            ps_u[:, pp * 130:pp * 130 + 65],
                            qphi_t[:, p, bs],
                            kvE_t[:, p, :],
                            start=True, stop=True)
                        nc.tensor.matmul(
                            ps_u[:, pp * 130 + 65:pp * 130 + 130],
                            qphi_t[:, p, bs],
                            kvO_t[:, p, :],
                            start=True, stop=True)
                    ps_u3 = ps_u[:].rearrange("p (c j) -> p c j", j=65)
                    d_t = rcp_pool.tile([128, 4], f32, tag="rcp")
                    nc.vector.tensor_scalar_add(d_t[:], ps_u3[:, :, 64], EPS)
                    r_t = rcp_pool.tile([128, 4], f32, tag="rcp")
                    nc.vector.reciprocal(r_t[:], d_t[:])
                    nc.vector.tensor_tensor(
                        out_t[:].rearrange("p (c d) -> p c d", d=64)[
                            :, half * 4:(half + 1) * 4, :],
                        ps_u3[:, :, 0:64],
                        r_t[:, :, None].to_broadcast((128, 4, 64)),
                        OP.mult,
                    )
                nc.sync.dma_start(out_d[b * 128:(b + 1) * 128, :], out_t[:])

    nc.compile()
    return nc


def _get_nc(nblocks=TOK // BLK):
    if nblocks not in _CACHE:
        _CACHE[nblocks] = _build(nblocks)
    return _CACHE[nblocks]


def _prep_shared(Wq, bq, Wk, bk, Wv, bv):
    bf = ml_dtypes.bfloat16
    wq = np.ascontiguousarray(
        Wq.reshape(6, 128, 512).transpose(1, 0, 2)).astype(bf)
    wk = np.ascontiguousarray(
        Wk.reshape(6, 128, 512).transpose(1, 0, 2)).astype(bf)
    wv = np.ascontiguousarray(
        Wv.reshape(4, 128, 512).transpose(1, 0, 2)).astype(bf)
    wk7 = np.zeros((128, 512), np.float32)
    wk7[0, :] = bk
    wk7 = wk7.astype(bf)
    bvb = np.ascontiguousarray(
        np.broadcast_to(bv.reshape(8, 64), (128, 8, 64))).astype(bf)
    qb = np.ascontiguousarray(np.stack(
        [(-bq).reshape(4, 128).T, (bq + 1.0).reshape(4, 128).T],
        axis=-1)).astype(np.float32)
    return wq, wk, wv, wk7, bvb, qb


def kernel(x, guidance, Wq, bq, Wk, bk, Wv, bv):
    from concourse.bass_utils import run_bass_kernel_spmd

    x = np.asarray(x, dtype=np.float32)
    guidance = np.asarray(guidance, dtype=np.float32)
    Wq = np.asarray(Wq, dtype=np.float32)
    bq = np.asarray(bq, dtype=np.float32)
    Wk = np.asarray(Wk, dtype=np.float32)
    bk = np.asarray(bk, dtype=np.float32)
    Wv = np.asarray(Wv, dtype=np.float32)
    bv = np.asarray(bv, dtype=np.float32)

    nc = _get_nc()
    wq, wk, wv, wk7, bvb, qb = _prep_shared(Wq, bq, Wk, bk, Wv, bv)
    bf = ml_dtypes.bfloat16

    in_maps = []
    for c in range(NCORES):
        xs = np.asarray(x[c * BSH:(c + 1) * BSH]).reshape(TOK, HID)
        gs = np.asarray(guidance[c * BSH:(c + 1) * BSH]).reshape(TOK, GUID)
        xg = np.concatenate([xs, gs], axis=1)
        xgT = np.ascontiguousarray(xg.T).astype(bf)
        in_maps.append({"xgT": xgT, "wq": wq, "wk": wk, "wv": wv,
                        "wk7": wk7, "bvb": bvb, "qb": qb})

    res = run_bass_kernel_spmd(nc, in_maps, core_ids=list(range(NCORES)))
    outs = [r["out"] for r in res.results]
    return np.concatenate(outs, axis=0).reshape(B, L, H * D).astype(np.float32)



# revision 13
# speedup vs baseline: 1.2060x; 1.2060x over previous
"""Trainium2 Bass kernel for LoFTR-style linear attention (nn_AttentionLayer).

Data-parallel over B=1024 across 8 NeuronCores (128 batches/core, no
collectives).  Q/K projections run in fp8(e4m3) DoubleRow mode (2 contraction
planes per pass = 2x bf16 throughput); V projection and the attention einsums
stay bf16.  fp8 on Q/K is accuracy-safe because the Z-normalization cancels
their quantization errors (measured 3.7e-3 vs 2e-2 budget); fp8 on V is not.

Key tricks:
  - xg scaled by SX=16 and weights by SW=256 before e4m3 quantization; the
    2^-12 descale is folded into ACT scale operands.
  - phi(x) = elu(x)+1 == max(x+1, min(exp(x), 1)): exp is monotone so the
    min clip replaces computing exp(min(x,0)) -- one Exp pass per tile.
  - K bias (bk+1) enters the fp8 matmul as two constant contraction rows
    (value 32) against w6=e4m3(128*(bk+1)) and its e4m3 residual w7, so the
    K PSUM holds 4096*(z+1) and neither ACT nor DVE needs a free-dim bias.
  - Q bias is per-partition (feature-on-partition layout): ACT Exp carries
    bq, ACT Identity carries bq+1.
  - V' = [V + bv | 1] per head so KV' = phi(K)^T V' yields KV and Ksum in
    one matmul; U = phi(Q) @ KV' yields output and normalizer together.
  - KV' staged into a single (128,4,130) tile with statically-zeroed dual
    quadrants; U runs one N=130 matmul per head-pair at full K=128.
  - No on-chip normalization: [U | den] is copied PSUM->SBUF and DMA'd to
    HBM; the host does out = U/(den+eps).  GPSIMD cannot touch PSUM, so
    this keeps the DVE/ACT PSUM-read load at its floor, and the Pool
    engine absorbs the SBUF-only bf16 min/max ops.
"""

import numpy as np
import ml_dtypes

NCORES = 8
B, L, HID, GUID, H, D = 1024, 128, 512, 256, 8, 64
KIN = HID + GUID          # 768
BSH = B // NCORES         # 128 batches per core
TOK = BSH * L             # 16384 tokens per core
BLK = 512                 # tokens per block (4 batches)
SX = 16.0                 # fp8 activation scale
SW = 256.0                # fp8 weight scale
DS = 1.0 / (SX * SW)      # 2^-12 descale
EPS = 1e-6

_CACHE = {}


def _build(nblocks):
    from contextlib import ExitStack
    import concourse.bass as bass
    import concourse.mybir as mybir
    import concourse.tile as tile
    from concourse import bacc

    f32 = mybir.dt.float32
    bf16 = mybir.dt.bfloat16
    fp8 = mybir.dt.float8e4
    AF = mybir.ActivationFunctionType
    OP = mybir.AluOpType
    DR = mybir.MatmulPerfMode.DoubleRow

    ntok = nblocks * BLK

    nc = bacc.Bacc("TRN2", target_bir_lowering=False, debug=False,
                   num_devices=NCORES)

    xg8T = nc.dram_tensor("xg8T", (KIN, ntok), fp8, kind="ExternalInput").ap()
    xvT = nc.dram_tensor("xvT", (HID, ntok), bf16, kind="ExternalInput").ap()
    wq_d = nc.dram_tensor("wq8", (128, 6, 512), fp8, kind="ExternalInput").ap()
    wk_d = nc.dram_tensor("wk8", (128, 8, 512), fp8, kind="ExternalInput").ap()
    wv_d = nc.dram_tensor("wv", (128, 4, 512), bf16, kind="ExternalInput").ap()
    bvb_d = nc.dram_tensor("bvb", (128, 8, 64), bf16, kind="ExternalInput").ap()
    qb_d = nc.dram_tensor("qb", (128, 4, 2), f32, kind="ExternalInput").ap()
    u_d = nc.dram_tensor("u", (ntok, 520), f32, kind="ExternalOutput").ap()

    with tile.TileContext(nc) as tc, ExitStack() as ctx:
        consts = ctx.enter_context(tc.tile_pool(name="consts", bufs=1))
        xv_pool = ctx.enter_context(tc.tile_pool(name="xv", bufs=3))
        qphi_pool = ctx.enter_context(tc.tile_pool(name="qphi", bufs=2))
        tmp_pool = ctx.enter_context(tc.tile_pool(name="tmp", bufs=6))
        kphi_pool = ctx.enter_context(tc.tile_pool(name="kphi", bufs=2))
        out_pool = ctx.enter_context(tc.tile_pool(name="outp", bufs=4))
        psq_pool = ctx.enter_context(tc.tile_pool(name="psq", bufs=2, space="PSUM"))
        pskv_pool = ctx.enter_context(tc.tile_pool(name="pskv", bufs=3, space="PSUM"))
        psatt_pool = ctx.enter_context(tc.tile_pool(name="psatt", bufs=1, space="PSUM"))
        psu_pool = ctx.enter_context(tc.tile_pool(name="psu", bufs=2, space="PSUM"))

        wq_t = consts.tile([128, 6, 512], fp8)
        wk_t = consts.tile([128, 8, 512], fp8)
        wv_t = consts.tile([128, 4, 512], bf16)
        bvb_t = consts.tile([128, 8, 64], bf16)
        qb_t = consts.tile([128, 4, 2], f32)
        neg1_t = consts.tile([128, 1], f32)
        nc.vector.memset(neg1_t[:], -1.0)

        for k in range(8):
            if k < 6:
                nc.sync.dma_start(wq_t[:, k, :], wq_d[:, k, :])
            nc.sync.dma_start(wk_t[:, k, :], wk_d[:, k, :])
            if k < 4:
                nc.sync.dma_start(wv_t[:, k, :], wv_d[:, k, :])
        nc.sync.dma_start(bvb_t[:], bvb_d[:])
        nc.sync.dma_start(qb_t[:], qb_d[:])

        # xg fp8 tiles: manual 3-buffer rotation; planes 6/7 are the constant
        # bias rows for the K projection (row 0 = 32, rest 0), set once.
        xg8 = [consts.tile([128, 8, 512], fp8, name=f"xg8_{i}") for i in range(3)]
        for i in range(3):
            nc.vector.memset(xg8[i][:, 6:8, :], 0.0)
            nc.vector.memset(xg8[i][0:1, 6:8, :], 32.0)

        # V' staging tiles with static ones column (the Ksum column of KV'),
        # and KV' tiles with statically-zeroed dual quadrants so U can run
        # full K=128 against [E|O] packed columns.
        vp = [consts.tile([128, 8, 65], bf16, name=f"vp{i}") for i in range(2)]
        kv2 = [consts.tile([128, 4, 130], bf16, name=f"kv2_{i}") for i in range(2)]
        for i in range(2):
            nc.vector.memset(vp[i][:, :, 64:65], 1.0)
            nc.vector.memset(kv2[i][64:128, :, 0:65], 0.0)
            nc.vector.memset(kv2[i][0:64, :, 65:130], 0.0)

        for j in range(nblocks):
            xg_t = xg8[j % 3]
            xgv = xg8T[:, j * BLK:(j + 1) * BLK].rearrange(
                "(ko p) n -> p ko n", p=128)
            for k in range(6):
                nc.sync.dma_start(xg_t[:, k, :], xgv[:, k, :])
            xv_t = xv_pool.tile([128, 4, 512], bf16, tag="xv")
            xvv = xvT[:, j * BLK:(j + 1) * BLK].rearrange(
                "(ko p) n -> p ko n", p=128)
            for k in range(4):
                nc.sync.dma_start(xv_t[:, k, :], xvv[:, k, :])

            # ---- Q projection (feature-on-partition, fp8 DoubleRow) + phi --
            qphi_t = qphi_pool.tile([128, 4, 512], bf16, tag="qphi")
            for m in range(4):
                ps = psq_pool.tile([128, 512], f32, tag="psq")
                for kp in range(3):
                    nc.tensor.matmul(
                        ps[:],
                        wq_t[:, 2 * kp:2 * kp + 2, m * 128:(m + 1) * 128],
                        xg_t[:, 2 * kp:2 * kp + 2, :],
                        start=(kp == 0), stop=(kp == 2), perf_mode=DR,
                    )
                e = tmp_pool.tile([128, 512], bf16, tag="tmp")
                nc.scalar.activation(e[:], ps[:], AF.Exp,
                                     bias=qb_t[:, m, 0:1], scale=DS)
                z1 = tmp_pool.tile([128, 512], bf16, tag="tmp")
                nc.scalar.activation(z1[:], ps[:], AF.Identity,
                                     bias=qb_t[:, m, 1:2], scale=DS)
                # phi(q) = max(z+1, min(exp(z), 1)) in one SBUF-bf16 DVE op
                nc.vector.scalar_tensor_tensor(
                    qphi_t[:, m, :], e[:], 1.0, z1[:], OP.min, OP.max)

            for bi in range(4):
                b = j * 4 + bi
                bs = slice(bi * 128, (bi + 1) * 128)

                # ---- K projection (token-on-partition, fp8 DoubleRow);
                # pair (6,7) carries the (bk+1) bias rows ----
                ps_k = pskv_pool.tile([128, 512], f32, tag="pskv")
                for kp in range(4):
                    nc.tensor.matmul(
                        ps_k[:],
                        xg_t[:, 2 * kp:2 * kp + 2, bs],
                        wk_t[:, 2 * kp:2 * kp + 2, :],
                        start=(kp == 0), stop=(kp == 3), perf_mode=DR,
                    )

                # ---- V projection (bf16) ----
                ps_v = pskv_pool.tile([128, 512], f32, tag="pskv")
                for k in range(4):
                    nc.tensor.matmul(ps_v[:], xv_t[:, k, bs], wv_t[:, k, :],
                                     start=(k == 0), stop=(k == 3))

                # ---- phi(K): PSUM holds 4096*(z+1) ----
                e = tmp_pool.tile([128, 512], bf16, tag="tmp")
                nc.scalar.activation(e[:], ps_k[:], AF.Exp,
                                     bias=neg1_t[:], scale=DS)
                t = tmp_pool.tile([128, 512], bf16, tag="tmp")
                nc.vector.tensor_scalar_min(t[:], e[:], 1.0)
                kphi_t = kphi_pool.tile([128, 512], bf16, tag="kphi")
                nc.vector.scalar_tensor_tensor(
                    kphi_t[:], ps_k[:], DS, t[:], OP.mult, OP.max)

                # ---- V' = [V + bv | 1] per head ----
                vp_t = vp[b % 2]
                nc.vector.tensor_tensor(
                    vp_t[:, :, 0:64],
                    ps_v[:].rearrange("p (h d) -> p h d", d=64),
                    bvb_t[:], OP.add)

                # ---- KV' = phi(K)^T @ V' : 2 heads per pair via col tiling --
                ps_kv_full = psatt_pool.tile([128, 512], f32, tag="psatt",
                                             name="ps_kv")
                ps_kv = ps_kv_full[:, :260]
                for p in range(4):
                    nc.tensor.matmul(
                        ps_kv[0:64, p * 65:(p + 1) * 65],
                        kphi_t[:, p * 128:p * 128 + 64],
                        vp_t[:, 2 * p, :],
                        start=True, stop=True, tile_position=(0, 0))
                    nc.tensor.matmul(
                        ps_kv[64:128, p * 65:(p + 1) * 65],
                        kphi_t[:, p * 128 + 64:(p + 1) * 128],
                        vp_t[:, 2 * p + 1, :],
                        start=True, stop=True, tile_position=(0, 64))
                kv2_t = kv2[b % 2]
                # split across ACT and DVE so the two copies run in parallel
                nc.scalar.copy(
                    kv2_t[0:64, :, 0:65],
                    ps_kv[0:64, :].rearrange("p (c j) -> p c j", j=65))
                nc.vector.tensor_copy(
                    kv2_t[64:128, :, 65:130],
                    ps_kv[64:128, :].rearrange("p (c j) -> p c j", j=65))

                # ---- U = phi(Q) @ KV' : one N=130 matmul per head pair;
                # [U | den] goes to HBM unnormalized (host divides) ----
                for half in range(2):
                    ps_u_full = psu_pool.tile([128, 512], f32, tag="psu",
                                              name="ps_u")
                    ps_u = ps_u_full[:, :260]
                    for pp in range(2):
                        p = half * 2 + pp
                        nc.tensor.matmul(
                            ps_u[:, pp * 130:(pp + 1) * 130],
                            qphi_t[:, p, bs],
                            kv2_t[:, p, :],
                            start=True, stop=True)
                    uo = out_pool.tile([128, 260], f32, tag="outp")
                    # alternate the PSUM->SBUF copy between ACT and DVE
                    if half == 0:
                        nc.scalar.copy(uo[:], ps_u[:])
                    else:
                        nc.vector.tensor_copy(uo[:], ps_u[:])
                    nc.sync.dma_start(
                        u_d[b * 128:(b + 1) * 128, half * 260:(half + 1) * 260],
                        uo[:])

    nc.compile()
    return nc


def _get_nc(nblocks=TOK // BLK):
    if nblocks not in _CACHE:
        _CACHE[nblocks] = _build(nblocks)
    return _CACHE[nblocks]


def _prep_shared(Wq, bq, Wk, bk, Wv, bv):
    bf = ml_dtypes.bfloat16
    f8 = ml_dtypes.float8_e4m3
    wq8 = np.ascontiguousarray(
        (Wq.reshape(6, 128, 512).transpose(1, 0, 2)) * SW).astype(f8)
    wk8 = np.zeros((128, 8, 512), f8)
    wk8[:, 0:6, :] = ((Wk.reshape(6, 128, 512).transpose(1, 0, 2)) * SW
                      ).astype(f8)
    # bias rows: 32*w6 + 32*w7 == 4096*(bk+1) with e4m3 residual correction
    w6 = (128.0 * (bk + 1.0)).astype(f8)
    w7 = ((4096.0 * (bk + 1.0) - 32.0 * w6.astype(np.float32)) / 32.0
          ).astype(f8)
    wk8[0, 6, :] = w6
    wk8[0, 7, :] = w7
    wv = np.ascontiguousarray(
        Wv.reshape(4, 128, 512).transpose(1, 0, 2)).astype(bf)
    bvb = np.ascontiguousarray(
        np.broadcast_to(bv.reshape(8, 64), (128, 8, 64))).astype(bf)
    qb = np.ascontiguousarray(np.stack(
        [bq.reshape(4, 128).T, (bq + 1.0).reshape(4, 128).T],
        axis=-1)).astype(np.float32)
    return wq8, wk8, wv, bvb, qb


def _prep_core(x_c, g_c):
    bf = ml_dtypes.bfloat16
    f8 = ml_dtypes.float8_e4m3
    xs = np.asarray(x_c).reshape(TOK, HID)
    gs = np.asarray(g_c).reshape(TOK, GUID)
    xg = np.concatenate([xs, gs], axis=1)
    xg8T = np.ascontiguousarray((xg * SX).T).astype(f8)
    xvT = np.ascontiguousarray(xs.T).astype(bf)
    return xg8T, xvT


def _finish(u):
    # u: (TOK, 520) = per token 2 halves x [4 heads x (64 out | den)]
    u = u.reshape(TOK, 8, 65)
    return u[:, :, 0:64] / (u[:, :, 64:65] + EPS)


def kernel(x, guidance, Wq, bq, Wk, bk, Wv, bv):
    from concourse.bass_utils import run_bass_kernel_spmd

    x = np.asarray(x, dtype=np.float32)
    guidance = np.asarray(guidance, dtype=np.float32)
    Wq = np.asarray(Wq, dtype=np.float32)
    bq = np.asarray(bq, dtype=np.float32)
    Wk = np.asarray(Wk, dtype=np.float32)
    bk = np.asarray(bk, dtype=np.float32)
    Wv = np.asarray(Wv, dtype=np.float32)
    bv = np.asarray(bv, dtype=np.float32)

    nc = _get_nc()
    wq8, wk8, wv, bvb, qb = _prep_shared(Wq, bq, Wk, bk, Wv, bv)

    in_maps = []
    for c in range(NCORES):
        xg8T, xvT = _prep_core(x[c * BSH:(c + 1) * BSH],
                               guidance[c * BSH:(c + 1) * BSH])
        in_maps.append({"xg8T": xg8T, "xvT": xvT, "wq8": wq8, "wk8": wk8,
                        "wv": wv, "bvb": bvb, "qb": qb})

    res = run_bass_kernel_spmd(nc, in_maps, core_ids=list(range(NCORES)))
    outs = [_finish(r["u"]) for r in res.results]
    return np.concatenate(outs, axis=0).reshape(B, L, H * D).astype(np.float32)
